# revision 10
# baseline (speedup 1.0000x reference)
"""Trainium2 Bass kernel for nn_LinearWithGroupedConv (out = x @ weight.T).

Full-input contract: kernel(x=[4,2048,4096] f32, weight=[4096,4096] f32)
-> [4,2048,4096] f32.

Strategy (tensor-parallel, column sharding per the hint):
  - out[s, o] = sum_k x[s, k] * weight[o, k];  S=8192 (4*2048), K=4096, O=4096.
  - Shard `weight` over out_feature across 8 cores (512 columns each),
    replicate x. Each core computes out_shard [8192, 512]; host concats.
  - On host: transpose x -> xT [K, S] and weight -> wT [K, O] so the
    contraction dim lands on SBUF partitions, and cast to fp16 (PSUM
    accumulation is fp32; fp16 keeps 10 mantissa bits -> rel err ~2e-4).
  - Per core: keep the full wT shard resident in SBUF ([128, 32, 512] fp16),
    stream xT in 4 MB chunks ([128, 32, 512] fp16), accumulate 32 matmuls
    (K-tiles) per 128-row output tile into one PSUM bank, copy to SBUF via
    DVE, DMA out.
"""

import ml_dtypes
import numpy as np

import concourse.bass as bass
import concourse.mybir as mybir
import concourse.tile as tile
from concourse import bacc
from concourse.bass_utils import run_bass_kernel_spmd

N_CORES = 8
S = 8192          # 4 * 2048 sequence rows
K = 4096          # in_feature (contraction)
O = 4096          # out_feature
O_SHARD = O // N_CORES          # 512
P = 128
K_TILES = K // P                # 32
S_CHUNK = 512                   # seq columns per streamed x chunk
S_SUB = S_CHUNK // P            # 4 psum tiles per chunk
N_CHUNKS = S // S_CHUNK         # 16

# "fp16": single-pass fp16 matmul (rel err ~2e-4)
# "split": 3-pass fp16 hi/lo split (rel err ~1e-5, 3x compute)
MODE = "fp16"
PROFILE = False          # test.py sets True to capture an NTFF trace
LAST_PROFILE = None      # BassKernelResults of the last run when PROFILE

_CACHE = {}


def _build_fp16(split: bool, dt16=mybir.dt.float16):
    nc = bacc.Bacc(None, target_bir_lowering=False)
    n_terms = 3 if split else 1

    xs = []
    ws = []
    if split:
        names = [("x_hi", "w_hi"), ("x_hi", "w_lo"), ("x_lo", "w_hi")]
        x_hi = nc.dram_tensor("x_hi", [K, S], dt16, kind="ExternalInput")
        x_lo = nc.dram_tensor("x_lo", [K, S], dt16, kind="ExternalInput")
        w_hi = nc.dram_tensor("w_hi", [K, O_SHARD], dt16, kind="ExternalInput")
        w_lo = nc.dram_tensor("w_lo", [K, O_SHARD], dt16, kind="ExternalInput")
        handles = {"x_hi": x_hi, "x_lo": x_lo, "w_hi": w_hi, "w_lo": w_lo}
        x_names = ["x_hi", "x_lo"]
        w_names = ["w_hi", "w_lo"]
    else:
        names = [("x", "w")]
        handles = {
            "x": nc.dram_tensor("x", [K, S], dt16, kind="ExternalInput"),
            "w": nc.dram_tensor("w", [K, O_SHARD], dt16, kind="ExternalInput"),
        }
        x_names = ["x"]
        w_names = ["w"]

    out = nc.dram_tensor("out", [S, O_SHARD], mybir.dt.float32, kind="ExternalOutput")

    with tile.TileContext(nc) as tc:
        with (
            tc.tile_pool(name="wpool", bufs=1) as wpool,
            tc.tile_pool(name="xpool", bufs=2) as xpool,
            tc.tile_pool(name="x0pool", bufs=1) as x0pool,
            tc.tile_pool(name="opool", bufs=4) as opool,
            tc.tile_pool(name="psum", bufs=4, space=bass.MemorySpace.PSUM) as psum,
        ):
            # Per-k-tile weight tiles + per-k first x chunk, interleaved, so
            # the first accumulation group starts after ~2 small DMAs instead
            # of two monolithic 4 MB loads (shrinks the kernel head).
            w_sb = {}   # wn -> list of [P, O_SHARD] tiles per k
            for wn in w_names:
                w_sb[wn] = [
                    wpool.tile([P, O_SHARD], dt16, tag=f"{wn}_{k}", name=f"w_sb_{wn}_{k}")
                    for k in range(K_TILES)
                ]
            x0_sb = {}  # xn -> list of [P, S_CHUNK] tiles per k (chunk 0)
            for xn in x_names:
                x0_sb[xn] = [
                    x0pool.tile([P, S_CHUNK], dt16, tag=f"{xn}0_{k}", name=f"x0_sb_{xn}_{k}")
                    for k in range(K_TILES)
                ]
            for k in range(K_TILES):
                for wn in w_names:
                    nc.sync.dma_start(
                        w_sb[wn][k][:],
                        handles[wn][k * P:(k + 1) * P, :],
                    )
                for xn in x_names:
                    nc.sync.dma_start(
                        x0_sb[xn][k][:],
                        handles[xn][k * P:(k + 1) * P, 0:S_CHUNK],
                    )

            for c in range(N_CHUNKS):
                x_sb = {}
                if c == 0:
                    def x_tile(xn, k, ss):
                        return x0_sb[xn][k][:, ss * P:(ss + 1) * P]
                else:
                    for xn in x_names:
                        x_sb[xn] = xpool.tile(
                            [P, K_TILES, S_CHUNK], dt16, tag=xn, name=f"x_sb_{xn}"
                        )
                        nc.sync.dma_start(
                            x_sb[xn][:],
                            handles[xn][:, c * S_CHUNK:(c + 1) * S_CHUNK].rearrange(
                                "(k p) s -> p k s", p=P
                            ),
                        )

                    def x_tile(xn, k, ss, x_sb=x_sb):
                        return x_sb[xn][:, k, ss * P:(ss + 1) * P]
                for ss in range(S_SUB):
                    pt = psum.tile([P, O_SHARD], mybir.dt.float32)
                    n_mms = n_terms * K_TILES
                    i = 0
                    for xn, wn in names:
                        for k in range(K_TILES):
                            nc.tensor.matmul(
                                pt[:],
                                x_tile(xn, k, ss),
                                w_sb[wn][k][:],
                                start=(i == 0),
                                stop=(i == n_mms - 1),
                            )
                            i += 1
                    o_sb = opool.tile([P, O_SHARD], mybir.dt.float32)
                    nc.vector.tensor_copy(o_sb[:], pt[:])
                    s0 = c * S_CHUNK + ss * P
                    nc.sync.dma_start(out[s0:s0 + P, :], o_sb[:])
    nc.compile()
    return nc


def _install_ntff_hook():
    """Register the axon NTFF profiling hook if the image's antenv lacks it.

    Only used when PROFILE=True (test harness); grading never hits this.
    """
    import sys
    import types

    if "antenv.axon_hooks" in sys.modules:
        return
    try:
        from trn_agent_boot.trn_boot import _ntff_profile_via_ctypes
    except ImportError:
        return
    try:
        hook = _ntff_profile_via_ctypes("/opt/axon/libaxon_pjrt.so")
    except OSError:
        return
    m = types.ModuleType("antenv.axon_hooks")
    m.get_axon_ntff_profile_hook = lambda: hook
    m.set_axon_ntff_profile_hook = lambda h: None
    sys.modules["antenv.axon_hooks"] = m


def _get_nc():
    key = MODE
    if key not in _CACHE:
        if MODE == "fp16":
            _CACHE[key] = _build_fp16(split=False)
        elif MODE == "bf16":
            _CACHE[key] = _build_fp16(split=False, dt16=mybir.dt.bfloat16)
        elif MODE == "split":
            _CACHE[key] = _build_fp16(split=True)
        else:
            raise ValueError(f"unknown MODE {MODE}")
    return _CACHE[key]


def kernel(x: np.ndarray, weight: np.ndarray) -> np.ndarray:
    global LAST_PROFILE
    b, s, k = x.shape
    assert (b * s, k) == (S, K) and weight.shape == (O, K)

    xT = np.ascontiguousarray(x.reshape(S, K).T)          # [K, S] f32
    wT = np.ascontiguousarray(weight.T)                   # [K, O] f32

    if MODE in ("fp16", "bf16"):
        np16 = np.float16 if MODE == "fp16" else ml_dtypes.bfloat16
        xT16 = xT.astype(np16)
        wT16 = wT.astype(np16)
        in_maps = [
            {"x": xT16, "w": np.ascontiguousarray(wT16[:, c * O_SHARD:(c + 1) * O_SHARD])}
            for c in range(N_CORES)
        ]
    else:
        x_hi = xT.astype(np.float16)
        x_lo = (xT - x_hi.astype(np.float32)).astype(np.float16)
        w_hi = wT.astype(np.float16)
        w_lo = (wT - w_hi.astype(np.float32)).astype(np.float16)
        in_maps = [
            {
                "x_hi": x_hi,
                "x_lo": x_lo,
                "w_hi": np.ascontiguousarray(w_hi[:, c * O_SHARD:(c + 1) * O_SHARD]),
                "w_lo": np.ascontiguousarray(w_lo[:, c * O_SHARD:(c + 1) * O_SHARD]),
            }
            for c in range(N_CORES)
        ]

    if PROFILE:
        _install_ntff_hook()
    nc = _get_nc()
    res = run_bass_kernel_spmd(
        nc,
        in_maps,
        core_ids=list(range(N_CORES)),
        trace=PROFILE,
        trace_cores=[0] if PROFILE else None,
    )
    LAST_PROFILE = res

    full = np.empty((S, O), dtype=np.float32)
    for c in range(N_CORES):
        full[:, c * O_SHARD:(c + 1) * O_SHARD] = res.results[c]["out"]
    return full.reshape(b, s, O)


# revision 14
# speedup vs baseline: 1.0081x; 1.0081x over previous
"""Trainium2 Bass kernel for nn_LinearWithGroupedConv (out = x @ weight.T).

Full-input contract: kernel(x=[4,2048,4096] f32, weight=[4096,4096] f32)
-> [4,2048,4096] f32.

Strategy (tensor-parallel, column sharding per the hint):
  - out[s, o] = sum_k x[s, k] * weight[o, k];  S=8192 (4*2048), K=4096, O=4096.
  - Shard `weight` over out_feature across 8 cores (512 columns each),
    replicate x. Each core computes out_shard [8192, 512]; host concats.
  - On host: transpose x -> xT [K, S] and weight -> wT [K, O] so the
    contraction dim lands on SBUF partitions, and cast to fp16 (PSUM
    accumulation is fp32; fp16 keeps 10 mantissa bits -> rel err ~2e-4).
  - Per core: keep the full wT shard resident in SBUF ([128, 32, 512] fp16),
    stream xT in 4 MB chunks ([128, 32, 512] fp16), accumulate 32 matmuls
    (K-tiles) per 128-row output tile into one PSUM bank, copy to SBUF via
    DVE, DMA out.
"""

import ml_dtypes
import numpy as np

import concourse.bass as bass
import concourse.mybir as mybir
import concourse.tile as tile
from concourse import bacc
from concourse.bass_utils import run_bass_kernel_spmd

N_CORES = 8
S = 8192          # 4 * 2048 sequence rows
K = 4096          # in_feature (contraction)
O = 4096          # out_feature
O_SHARD = O // N_CORES          # 512
P = 128
K_TILES = K // P                # 32
S_CHUNK = 512                   # seq columns per streamed x chunk
S_SUB = S_CHUNK // P            # 4 psum tiles per chunk
N_CHUNKS = S // S_CHUNK         # 16

# "fp16": single-pass fp16 matmul (rel err ~2e-4)
# "split": 3-pass fp16 hi/lo split (rel err ~1e-5, 3x compute)
MODE = "fp16"
PROFILE = False          # test.py sets True to capture an NTFF trace
LAST_PROFILE = None      # BassKernelResults of the last run when PROFILE

_CACHE = {}


def _build_fp16(split: bool, dt16=mybir.dt.float16):
    nc = bacc.Bacc(None, target_bir_lowering=False)
    n_terms = 3 if split else 1

    xs = []
    ws = []
    if split:
        names = [("x_hi", "w_hi"), ("x_hi", "w_lo"), ("x_lo", "w_hi")]
        x_hi = nc.dram_tensor("x_hi", [K, S], dt16, kind="ExternalInput")
        x_lo = nc.dram_tensor("x_lo", [K, S], dt16, kind="ExternalInput")
        w_hi = nc.dram_tensor("w_hi", [K, O_SHARD], dt16, kind="ExternalInput")
        w_lo = nc.dram_tensor("w_lo", [K, O_SHARD], dt16, kind="ExternalInput")
        handles = {"x_hi": x_hi, "x_lo": x_lo, "w_hi": w_hi, "w_lo": w_lo}
        x_names = ["x_hi", "x_lo"]
        w_names = ["w_hi", "w_lo"]
    else:
        names = [("x", "w")]
        handles = {
            "x": nc.dram_tensor("x", [K, S], dt16, kind="ExternalInput"),
            "w": nc.dram_tensor("w", [K, O_SHARD], dt16, kind="ExternalInput"),
        }
        x_names = ["x"]
        w_names = ["w"]

    out = nc.dram_tensor("out", [S, O_SHARD], mybir.dt.float32, kind="ExternalOutput")

    with tile.TileContext(nc) as tc:
        with (
            tc.tile_pool(name="wpool", bufs=1) as wpool,
            tc.tile_pool(name="xpool", bufs=2) as xpool,
            tc.tile_pool(name="x0pool", bufs=1) as x0pool,
            tc.tile_pool(name="opool", bufs=4) as opool,
            tc.tile_pool(name="psum", bufs=8, space=bass.MemorySpace.PSUM) as psum,
        ):
            # Per-k-tile weight tiles + per-k first x chunk, interleaved, so
            # the first accumulation group starts after ~2 small DMAs instead
            # of two monolithic 4 MB loads (shrinks the kernel head).
            w_sb = {}   # wn -> list of [P, O_SHARD] tiles per k
            for wn in w_names:
                w_sb[wn] = [
                    wpool.tile([P, O_SHARD], dt16, tag=f"{wn}_{k}", name=f"w_sb_{wn}_{k}")
                    for k in range(K_TILES)
                ]
            x0_sb = {}  # xn -> list of [P, S_CHUNK] tiles per k (chunk 0)
            for xn in x_names:
                x0_sb[xn] = [
                    x0pool.tile([P, S_CHUNK], dt16, tag=f"{xn}0_{k}", name=f"x0_sb_{xn}_{k}")
                    for k in range(K_TILES)
                ]
            # w on the SP HWDGE ring, x on the ACT HWDGE ring -> the two
            # streams transfer concurrently and stay ahead of the k-outer
            # matmul order below.
            for k in range(K_TILES):
                for wn in w_names:
                    nc.sync.dma_start(
                        w_sb[wn][k][:],
                        handles[wn][k * P:(k + 1) * P, :],
                    )
                for xn in x_names:
                    nc.scalar.dma_start(
                        x0_sb[xn][k][:],
                        handles[xn][k * P:(k + 1) * P, 0:S_CHUNK],
                    )

            for c in range(N_CHUNKS):
                x_sb = {}
                if c == 0:
                    def x_tile(xn, k, ss):
                        return x0_sb[xn][k][:, ss * P:(ss + 1) * P]
                else:
                    for xn in x_names:
                        x_sb[xn] = xpool.tile(
                            [P, K_TILES, S_CHUNK], dt16, tag=xn, name=f"x_sb_{xn}"
                        )
                        nc.scalar.dma_start(
                            x_sb[xn][:],
                            handles[xn][:, c * S_CHUNK:(c + 1) * S_CHUNK].rearrange(
                                "(k p) s -> p k s", p=P
                            ),
                        )

                    def x_tile(xn, k, ss, x_sb=x_sb):
                        return x_sb[xn][:, k, ss * P:(ss + 1) * P]
                # k-outer, ss-inner: 4 PSUM accumulation groups run in
                # parallel, so k-tile k isn't needed until ~k*0.86us — the
                # streamed chunk-0 loads stay ahead of consumption.
                pts = [
                    psum.tile([P, O_SHARD], mybir.dt.float32, tag="pt", name=f"pt{ss}")
                    for ss in range(S_SUB)
                ]
                n_k = n_terms * K_TILES
                ki = 0
                for xn, wn in names:
                    for k in range(K_TILES):
                        for ss in range(S_SUB):
                            nc.tensor.matmul(
                                pts[ss][:],
                                x_tile(xn, k, ss),
                                w_sb[wn][k][:],
                                start=(ki == 0),
                                stop=(ki == n_k - 1),
                            )
                        ki += 1
                for ss in range(S_SUB):
                    o_sb = opool.tile([P, O_SHARD], mybir.dt.float32)
                    nc.vector.tensor_copy(o_sb[:], pts[ss][:])
                    s0 = c * S_CHUNK + ss * P
                    nc.sync.dma_start(out[s0:s0 + P, :], o_sb[:])
    nc.compile()
    return nc


def _install_ntff_hook():
    """Register the axon NTFF profiling hook if the image's antenv lacks it.

    Only used when PROFILE=True (test harness); grading never hits this.
    """
    import sys
    import types

    if "antenv.axon_hooks" in sys.modules:
        return
    try:
        from trn_agent_boot.trn_boot import _ntff_profile_via_ctypes
    except ImportError:
        return
    try:
        hook = _ntff_profile_via_ctypes("/opt/axon/libaxon_pjrt.so")
    except OSError:
        return
    m = types.ModuleType("antenv.axon_hooks")
    m.get_axon_ntff_profile_hook = lambda: hook
    m.set_axon_ntff_profile_hook = lambda h: None
    sys.modules["antenv.axon_hooks"] = m


def _get_nc():
    key = MODE
    if key not in _CACHE:
        if MODE == "fp16":
            _CACHE[key] = _build_fp16(split=False)
        elif MODE == "bf16":
            _CACHE[key] = _build_fp16(split=False, dt16=mybir.dt.bfloat16)
        elif MODE == "split":
            _CACHE[key] = _build_fp16(split=True)
        else:
            raise ValueError(f"unknown MODE {MODE}")
    return _CACHE[key]


def kernel(x: np.ndarray, weight: np.ndarray) -> np.ndarray:
    global LAST_PROFILE
    b, s, k = x.shape
    assert (b * s, k) == (S, K) and weight.shape == (O, K)

    xT = np.ascontiguousarray(x.reshape(S, K).T)          # [K, S] f32
    wT = np.ascontiguousarray(weight.T)                   # [K, O] f32

    if MODE in ("fp16", "bf16"):
        np16 = np.float16 if MODE == "fp16" else ml_dtypes.bfloat16
        xT16 = xT.astype(np16)
        wT16 = wT.astype(np16)
        in_maps = [
            {"x": xT16, "w": np.ascontiguousarray(wT16[:, c * O_SHARD:(c + 1) * O_SHARD])}
            for c in range(N_CORES)
        ]
    else:
        x_hi = xT.astype(np.float16)
        x_lo = (xT - x_hi.astype(np.float32)).astype(np.float16)
        w_hi = wT.astype(np.float16)
        w_lo = (wT - w_hi.astype(np.float32)).astype(np.float16)
        in_maps = [
            {
                "x_hi": x_hi,
                "x_lo": x_lo,
                "w_hi": np.ascontiguousarray(w_hi[:, c * O_SHARD:(c + 1) * O_SHARD]),
                "w_lo": np.ascontiguousarray(w_lo[:, c * O_SHARD:(c + 1) * O_SHARD]),
            }
            for c in range(N_CORES)
        ]

    if PROFILE:
        _install_ntff_hook()
    nc = _get_nc()
    res = run_bass_kernel_spmd(
        nc,
        in_maps,
        core_ids=list(range(N_CORES)),
        trace=PROFILE,
        trace_cores=[0] if PROFILE else None,
    )
    LAST_PROFILE = res

    full = np.empty((S, O), dtype=np.float32)
    for c in range(N_CORES):
        full[:, c * O_SHARD:(c + 1) * O_SHARD] = res.results[c]["out"]
    return full.reshape(b, s, O)
